# revision 18
# baseline (speedup 1.0000x reference)
"""Causal self-attention (B=2, T=2048, C=1024, H=16) on 8 TRN2 NeuronCores.

Sharding: data parallel over batch (2) x tensor parallel over heads (4 groups
of 4 heads). Each core computes qkv + attention for its 4 heads of one batch,
normalized attention outputs are AllGathered (one AG per 512-row t-chunk, the
last chunk per-pair) within each batch group of 4 cores, and each core then
computes a 256-column slice of the output projection. The host concatenates
the column slices (pure gather, no reduction).

v4: the causal boundary mask is applied additively (-1e8) by the tensor
engine into the scores PSUM via an identity matmul, removing the
exp->vector->AV serialization; startup loads at half-tile granularity across
three trigger queues; qkv/proj matmuls pumped into the exp-rate-bound
attention loop as fillers; per-pair AllGather for the tail chunk with
pair-major proj so the tail chases the final collective.
"""
import numpy as np
import ml_dtypes

import concourse.bass as bass
import concourse.tile as tile
from concourse import bacc, mybir
from concourse.bass_utils import run_bass_kernel_spmd

BF16 = ml_dtypes.bfloat16

B, T, C, H, D = 2, 2048, 1024, 16, 64
NCORES = 8
HPC = 4              # heads per core
FQK = 2 * HPC * D    # 512 rows of q+k per core
FV = HPC * D         # 256 rows of v per core
CT = C // 128        # 8 contraction tiles
TC5 = T // 512       # 4 t-chunks of 512
SB = T // 128        # 16 s-blocks of 128
SCALE = 1.0 / 8.0    # 1/sqrt(D)

_CACHE = {}


def _build_kernel():
    nc = bacc.Bacc("TRN2", target_bir_lowering=False, debug=False,
                   num_devices=NCORES)
    dt = mybir.dt
    f32, bf16 = dt.float32, dt.bfloat16

    xT = nc.dram_tensor("xT", [C, T], bf16, kind="ExternalInput").ap()
    wqkT = nc.dram_tensor("wqkT", [C, FQK], bf16, kind="ExternalInput").ap()
    wvT = nc.dram_tensor("wvT", [C, FV], bf16, kind="ExternalInput").ap()
    wpT = nc.dram_tensor("wpT", [C, FV], bf16, kind="ExternalInput").ap()
    bqk = nc.dram_tensor("bqk", [FQK, 1], f32, kind="ExternalInput").ap()
    bv = nc.dram_tensor("bv", [1, FV], bf16, kind="ExternalInput").ap()
    bp = nc.dram_tensor("bp", [1, FV], bf16, kind="ExternalInput").ap()
    negm = nc.dram_tensor("negm", [128, 128], bf16, kind="ExternalInput").ap()
    ident = nc.dram_tensor("ident", [128, 128], bf16, kind="ExternalInput").ap()
    onesr = nc.dram_tensor("onesr", [1, 64], mybir.dt.float32r,
                           kind="ExternalInput").ap()
    out = nc.dram_tensor("out", [T, FV], f32, kind="ExternalOutput").ap()

    with tile.TileContext(nc) as tc:
        with (
            tc.tile_pool(name="persist", bufs=1) as pp,
            tc.tile_pool(name="work", bufs=4) as wp,
            tc.tile_pool(name="attT", bufs=6) as ap_pool,
            tc.tile_pool(name="outsb", bufs=3) as op,
            tc.tile_pool(name="ps_qk", bufs=2, space="PSUM") as ps_qk,
            tc.tile_pool(name="ps_y", bufs=2, space="PSUM") as ps_y,
            tc.tile_pool(name="ps_f", bufs=2, space="PSUM") as ps_f,
            tc.tile_pool(name="dram", bufs=1, space="DRAM") as dram,
        ):
            # local constants first so the PE warmup can start immediately
            ones16 = pp.tile([1, 128], bf16, tag="ones16")
            nc.vector.memset(ones16[:], 1.0)
            junk_b = pp.tile([1, 512], bf16, tag="junk_b")
            nc.vector.memset(junk_b[:], 1.0)
            junk_f = pp.tile([1, 64], f32, tag="junk_f")
            nc.vector.memset(junk_f[:], 0.0)
            junk_o = pp.tile([1, 64], bf16, tag="junk_o")

            wqk_s = pp.tile([128, CT, FQK], bf16, tag="wqk")
            xT_s = pp.tile([128, CT, T], bf16, tag="xT")
            wv_s = pp.tile([128, CT, FV], bf16, tag="wv")
            wp_s = pp.tile([128, CT, FV], bf16, tag="wp")
            bqk_s = pp.tile([128, 4], f32, tag="bqk")
            bv_s = pp.tile([1, FV], bf16, tag="bv")
            bp_s = pp.tile([1, FV], bf16, tag="bp")
            negm_s = pp.tile([128, 128], bf16, tag="negm")
            ident_s = pp.tile([128, 128], bf16, tag="ident")
            ones32 = pp.tile([1, 64], mybir.dt.float32r, tag="ones32")

            xTr = xT.rearrange("(n p) t -> p n t", p=128)
            wqkr = wqkT.rearrange("(n p) f -> p n f", p=128)
            wvr = wvT.rearrange("(n p) f -> p n f", p=128)
            wpr = wpT.rearrange("(n p) f -> p n f", p=128)

            # critical loads at half-tile granularity: each dma_start lands
            # on one ~13GB/s DMA engine, so more triggers = more parallelism.
            # sync: wqk halves; scalar: wv first (v chains follow qkT by only
            # ~10us), then x chunk 0, then wp; gpsimd (SWDGE): small consts,
            # then the later x chunks (naturally delayed behind the
            # startup-critical traffic by the ~1us/trigger SWDGE issue rate).
            for ci in range(0, CT, 2):
                nc.scalar.dma_start(wv_s[:, ci:ci + 2, :], wvr[:, ci:ci + 2, :])
            for ci in range(CT):
                nc.sync.dma_start(wqk_s[:, ci, 0:256], wqkr[:, ci, 0:256])
                nc.sync.dma_start(wqk_s[:, ci, 256:512], wqkr[:, ci, 256:512])
                nc.scalar.dma_start(xT_s[:, ci, 0:256], xTr[:, ci, 0:256])
                nc.scalar.dma_start(xT_s[:, ci, 256:512], xTr[:, ci, 256:512])
            nc.gpsimd.dma_start(bqk_s[:], bqk.rearrange("(n p) o -> p (n o)", p=128))
            nc.gpsimd.dma_start(bv_s[:], bv[:])
            nc.gpsimd.dma_start(bp_s[:], bp[:])
            nc.gpsimd.dma_start(ones32[:], onesr[:])
            nc.gpsimd.dma_start(negm_s[:], negm[:])
            nc.gpsimd.dma_start(ident_s[:], ident[:])
            for t5x in range(1, TC5):
                for ci in range(CT):
                    nc.gpsimd.dma_start(xT_s[:, ci, t5x * 512:(t5x + 1) * 512],
                                        xTr[:, ci, t5x * 512:(t5x + 1) * 512])
            for ci in range(0, CT, 4):
                nc.scalar.dma_start(wp_s[:, ci:ci + 4, :], wpr[:, ci:ci + 4, :])

            # ---- warm the collective stream: a tiny AllGather issued at
            #      startup absorbs the ~11us first-collective latency (and
            #      queues behind the runtime's skew barrier) ----
            agw_in = dram.tile([1, 64], bf16, tag="agw_i", name="agw_i")
            agw_out = dram.tile([4, 64], bf16, tag="agw_o", name="agw_o")
            nc.gpsimd.dma_start(agw_in[:], bv[0:1, 0:64])
            nc.gpsimd.collective_compute(
                "AllGather", mybir.AluOpType.bypass,
                replica_groups=[[0, 1, 2, 3], [4, 5, 6, 7]],
                ins=[agw_in[:].opt()],
                outs=[agw_out[:].opt()],
            )

            # ---- PE p-state warmup + ACT table warm ----
            psw = ps_f.tile([128, 512], f32, tag="f", name="ps_warm")
            for i in range(6):
                nc.tensor.matmul(psw[:], ones16[0:1, :], junk_b[0:1, :],
                                 start=(i == 0), stop=(i == 5))
            nc.scalar.activation(junk_o[:], junk_f[:],
                                 mybir.ActivationFunctionType.Exp, scale=SCALE)

            bv_bc = pp.tile([128, FV], f32, tag="bv_bc")
            bp_bc = pp.tile([128, FV], f32, tag="bp_bc")

            def gen_bias_bc():
                # broadcast bias rows to all 128 partitions (K=1 matmuls)
                for row, bc_t in ((bv_s, bv_bc), (bp_s, bp_bc)):
                    psb = ps_f.tile([128, 512], f32, tag="f", name="ps_bias")
                    nc.tensor.matmul(psb[:, 0:FV], ones16[0:1, :], row[0:1, :],
                                     start=True, stop=True)
                    nc.vector.tensor_copy(bc_t[:], psb[:, 0:FV])
                    yield

            # ---- qkT / v storage ----
            qkT_s = [pp.tile([128, T], bf16, tag=f"qkT{fc}", name=f"qkT{fc}")
                     for fc in range(4)]
            vaug = pp.tile([128, SB, HPC * 65], bf16, tag="vaug")
            # only the per-head "ones" columns (col 64 of each 65) need init
            vones = vaug[:].rearrange("p s (h x) -> p s h x", h=HPC)[:, :, :, 64:65]
            nc.vector.memset(vones, 1.0)

            # ---- filler emitters: single-matmul granularity ----
            def gen_qkT_tile(t5, fc):
                ps = ps_f.tile([128, 512], f32, tag="f", name="ps_qkv")
                if t5 == 0:
                    # chunk 0 streams in as [128,256] halves; match that.
                    # start=True zeroes the WHOLE psum bank, so only the very
                    # first matmul may carry it — the second region then
                    # accumulates onto the zeroed bank.
                    for j, (ci, th) in enumerate(
                            (ci, th) for ci in range(CT) for th in range(2)):
                        nc.tensor.matmul(
                            ps[:, th * 256:(th + 1) * 256],
                            wqk_s[:, ci, fc * 128:(fc + 1) * 128],
                            xT_s[:, ci, th * 256:(th + 1) * 256],
                            start=(j == 0), stop=(ci == CT - 1),
                            skip_group_check=True,
                        )
                        if j < 2 * CT - 1:
                            yield
                else:
                    for ci in range(CT):
                        nc.tensor.matmul(
                            ps[:],
                            wqk_s[:, ci, fc * 128:(fc + 1) * 128],
                            xT_s[:, ci, t5 * 512:(t5 + 1) * 512],
                            start=(ci == 0), stop=(ci == CT - 1),
                        )
                        if ci < CT - 1:
                            yield
                nc.vector.tensor_scalar_add(
                    qkT_s[fc][:, t5 * 512:(t5 + 1) * 512], ps[:],
                    bqk_s[:, fc:fc + 1],
                )
                yield

            def gen_v_tile(tb):
                ps = ps_f.tile([128, 512], f32, tag="f", name="ps_v")
                for ci in range(CT):
                    nc.tensor.matmul(
                        ps[:, 0:FV],
                        xT_s[:, ci, tb * 128:(tb + 1) * 128],
                        wv_s[:, ci, :],
                        start=(ci == 0), stop=(ci == CT - 1),
                    )
                    if ci < CT - 1:
                        yield
                dst = vaug[:, tb, :].rearrange("p (h x) -> p h x", h=HPC)[:, :, 0:64]
                src = ps[:, 0:FV].rearrange("p (h x) -> p h x", h=HPC)
                bias = bv_bc[:].rearrange("p (h x) -> p h x", h=HPC)
                nc.vector.scalar_tensor_tensor(
                    dst, src, 1.0, bias,
                    op0=mybir.AluOpType.mult, op1=mybir.AluOpType.add,
                )
                yield

            yf = {}

            def gen_proj_chunk(t5):
                # one psum chain per 128-row output tile; the tail chunk runs
                # pair-0 contraction tiles first so only the back half of each
                # chain waits on AG(3,1)
                yft = yf[t5]
                ci_order = (0, 2, 4, 6, 1, 3, 5, 7) if t5 == 3 else range(CT)
                for tq in range(4):
                    pso = ps_f.tile([128, 512], f32, tag="f", name="ps_o")
                    for j, ci in enumerate(ci_order):
                        par, cc = ci % 2, ci // 2
                        nc.tensor.matmul(
                            pso[:, 0:FV],
                            yft[:, cc, par * 512 + tq * 128:
                                par * 512 + (tq + 1) * 128],
                            wp_s[:, ci, :],
                            start=(j == 0), stop=(j == CT - 1),
                            skip_group_check=True,
                        )
                        if j < CT - 1:
                            yield
                    tb = t5 * 4 + tq
                    osb = op.tile([128, FV], f32, tag="osb", name="osb")
                    nc.vector.tensor_add(osb[:], pso[:, 0:FV], bp_bc[:])
                    nc.sync.dma_start(out[tb * 128:(tb + 1) * 128, :], osb[:])
                    yield

            fillers = []        # FIFO of single-matmul generators

            def pump(n=1):
                while n > 0 and fillers:
                    try:
                        next(fillers[0])
                        n -= 1
                    except StopIteration:
                        fillers.pop(0)

            def drain():
                while fillers:
                    pump(64)

            def drain_gens(gens):
                while any(g in fillers for g in gens):
                    pump(8)

            # ---- all qkv up front: the attention loop is exp-rate-bound
            # and can only absorb ~1.5 filler matmuls per iteration, so the
            # qkv work runs as one solid tensor block here (overlapping the
            # input DMA) and attention then runs gapless with proj fillers
            fillers.append(gen_bias_bc())
            for t5x in range(TC5):
                for fc in range(4):
                    fillers.append(gen_qkT_tile(t5x, fc))
                for tb in range(4 * t5x, 4 * t5x + 4):
                    fillers.append(gen_v_tile(tb))
            drain()

            # ---- attention loop, t-chunk major; AG + proj pipelined ----
            ag_in, ag_out = {}, {}

            def emit_attention(t5, pair):
                live = 4 * (t5 + 1)
                q_fc, k_fc = pair, 2 + pair
                ypsA = ps_y.tile([65, 512], f32, tag="y", name="ypsA")
                ypsB = ps_y.tile([65, 512], f32, tag="y", name="ypsB")
                for sb in range(live):
                    off = sb * 128 - t5 * 512
                    w = 512 - max(off, 0)     # live columns of this t-chunk
                    ps = ps_qk.tile([128, 1024], f32, tag="qk", name="ps_s")
                    bdry = off >= 0
                    for hh in range(2):
                        lo, hi = 64 * hh, 64 * (hh + 1)
                        nc.tensor.matmul(
                            ps[:, hh * 512 + 512 - w:(hh + 1) * 512],
                            qkT_s[k_fc][lo:hi, sb * 128:(sb + 1) * 128],
                            qkT_s[q_fc][lo:hi,
                                        (t5 + 1) * 512 - w:(t5 + 1) * 512],
                            start=True, stop=not bdry,
                            skip_group_check=True,
                        )
                    if bdry:
                        # additive causal mask (-1e8 above the diagonal) into
                        # the boundary 128 columns, via identity matmul
                        o = max(off, 0)
                        for hh in range(2):
                            nc.tensor.matmul(
                                ps[:, hh * 512 + o:hh * 512 + o + 128],
                                ident_s[:], negm_s[:],
                                start=False, stop=True,
                                skip_group_check=True,
                            )
                    a = ap_pool.tile([128, 1024], bf16, tag="attT",
                                     name="attT")
                    av = a[:].rearrange("p (g x) -> p g x", g=2)
                    pv = ps[:].rearrange("p (g x) -> p g x", g=2)
                    if off > 0:
                        nc.scalar.activation(
                            av[:, :, off:512], pv[:, :, off:512],
                            mybir.ActivationFunctionType.Exp, scale=SCALE,
                        )
                    else:
                        nc.scalar.activation(
                            a[:], ps[:],
                            mybir.ActivationFunctionType.Exp, scale=SCALE,
                        )
                    pump(1 if (off > 0 or sb % 2) else 2)
                    o = max(off, 0)
                    for hh, yps in ((0, ypsA), (1, ypsB)):
                        h = pair * 2 + hh
                        nc.tensor.matmul(
                            yps[:, o:512],
                            vaug[:, sb, h * 65:(h + 1) * 65],
                            a[:, hh * 512 + o:(hh + 1) * 512],
                            start=(sb == 0), stop=(sb == live - 1),
                            skip_group_check=True,
                        )
                # normalize: y / denom (denom = row 64 via ones column)
                for hh, yps in ((0, ypsA), (1, ypsB)):
                    den = wp.tile([1, 512], mybir.dt.float32r, tag="den",
                                  name="den")
                    nc.vector.tensor_copy(den[:], yps[64:65, :])
                    bc = ps_f.tile([128, 512], f32, tag="f", name="bc")
                    nc.tensor.matmul(bc[0:64, :], ones32[0:1, :], den[:],
                                     start=True, stop=True)
                    r = wp.tile([64, 512], f32, tag="recip", name="recip")
                    nc.vector.reciprocal_approx_fast(r[:], bc[0:64, :])
                    yn = wp.tile([64, 512], bf16, tag="yn", name="yn")
                    nc.vector.tensor_mul(yn[:], yps[0:64, :], r[:])
                    if t5 == 3:
                        nc.sync.dma_start(
                            ag_in[(3, pair)][hh * 64:(hh + 1) * 64, 0:256],
                            yn[:, 0:256])
                        nc.scalar.dma_start(
                            ag_in[(3, pair)][hh * 64:(hh + 1) * 64, 256:512],
                            yn[:, 256:512])
                    else:
                        nc.sync.dma_start(
                            ag_in[t5][hh * 64:(hh + 1) * 64,
                                      pair * 512:(pair + 1) * 512], yn[:])

            def trigger_ag(key, width):
                ag_out[key] = dram.tile([512, width], bf16,
                                        tag=f"agout{key}", name=f"agout{key}")
                nc.gpsimd.collective_compute(
                    "AllGather", mybir.AluOpType.bypass,
                    replica_groups=[[0, 1, 2, 3], [4, 5, 6, 7]],
                    ins=[ag_in[key][:].opt()],
                    outs=[ag_out[key][:].opt()],
                )

            def pull_yf(t5, eng):
                # [128,512] pieces (~10us each on one DMA engine, parallel
                # across engines); emitted as soon as AG(t5) is sure to finish
                # before anything queued behind these on `eng` is needed
                for cc in range(4):
                    for ph in range(2):
                        eng.dma_start(
                            yf[t5][:, cc, ph * 512:(ph + 1) * 512],
                            ag_out[t5][cc * 128:(cc + 1) * 128,
                                       ph * 512:(ph + 1) * 512])

            for t5 in range(TC5):
                yf[t5] = pp.tile([128, 4, 1024], bf16, tag="yf",
                                 bufs=3, name=f"yf{t5}")
                if t5 == 3:
                    for pair in range(2):
                        ag_in[(3, pair)] = dram.tile(
                            [128, 512], bf16, tag=f"agin3_{pair}",
                            name=f"agin3_{pair}")
                else:
                    ag_in[t5] = dram.tile([128, 1024], bf16, tag=f"agin{t5}",
                                          name=f"agin{t5}")
                if t5 == 2:
                    pull_yf(0, nc.sync)       # AG(0) long done
                if t5 == 3:
                    fillers.append(gen_proj_chunk(1))
                emit_attention(t5, 0)
                if t5 == 2:
                    pull_yf(1, nc.sync)       # AG(1) done ~20us ago
                    fillers.append(gen_proj_chunk(0))
                if t5 == 3:
                    trigger_ag((3, 0), 512)
                    pull_yf(2, nc.gpsimd)     # AG(2) completed during pair 0
                    fillers.append(gen_proj_chunk(2))
                emit_attention(t5, 1)
                if t5 == 3:
                    trigger_ag((3, 1), 512)
                else:
                    trigger_ag(t5, 1024)
                drain()
            # tail: pair-0 pull on gpsimd (AG(3,0) done during pair-1
            # attention); pair-1 in 64KB pieces split across sync+scalar
            # (both idle at the tail), chasing AG(3,1)
            for cc in range(4):
                nc.gpsimd.dma_start(yf[3][:, cc, 0:512],
                                    ag_out[(3, 0)][cc * 128:(cc + 1) * 128, :])
            for cc in range(4):
                for ph, eng in ((0, nc.sync), (1, nc.scalar)):
                    eng.dma_start(
                        yf[3][:, cc, 512 + ph * 256:512 + (ph + 1) * 256],
                        ag_out[(3, 1)][cc * 128:(cc + 1) * 128,
                                       ph * 256:(ph + 1) * 256])
            fillers.append(gen_proj_chunk(3))
            drain()

    nc.compile()
    return nc


def _shard_inputs(x, w_attn, b_attn, w_proj, b_proj):
    negm = np.zeros((128, 128), dtype=BF16)
    for s in range(128):
        negm[s, :s] = -800.0   # -100 after scale: exp()->0 within table range
    ident = np.eye(128, dtype=BF16)

    in_maps = []
    for core in range(NCORES):
        b, hg = core // 4, core % 4
        r0 = hg * HPC * D          # first q/k/v row offset within each 1024
        r1 = r0 + HPC * D
        wqk = np.concatenate([w_attn[r0:r1, :], w_attn[C + r0:C + r1, :]], 0)
        in_maps.append({
            "xT": np.ascontiguousarray(x[b].T).astype(BF16),
            "wqkT": np.ascontiguousarray(wqk.T).astype(BF16),
            "wvT": np.ascontiguousarray(w_attn[2 * C + r0:2 * C + r1, :].T).astype(BF16),
            "wpT": np.ascontiguousarray(w_proj[r0:r1, :].T).astype(BF16),
            "bqk": np.concatenate([b_attn[r0:r1], b_attn[C + r0:C + r1]])
                     .reshape(FQK, 1).astype(np.float32),
            "bv": b_attn[2 * C + r0:2 * C + r1].reshape(1, FV).astype(BF16),
            "bp": b_proj[r0:r1].reshape(1, FV).astype(BF16),
            "negm": negm,
            "ident": ident,
            "onesr": np.ones((1, 64), dtype=np.float32),
        })
    return in_maps


def kernel(x, w_attn, b_attn, w_proj, b_proj, _trace=False, _trace_kwargs=None):
    x = np.asarray(x, dtype=np.float32)
    w_attn = np.asarray(w_attn, dtype=np.float32)
    b_attn = np.asarray(b_attn, dtype=np.float32)
    w_proj = np.asarray(w_proj, dtype=np.float32)
    b_proj = np.asarray(b_proj, dtype=np.float32)

    if "nc" not in _CACHE:
        _CACHE["nc"] = _build_kernel()
    nc = _CACHE["nc"]

    in_maps = _shard_inputs(x, w_attn, b_attn, w_proj, b_proj)
    res = run_bass_kernel_spmd(nc, in_maps, core_ids=list(range(NCORES)),
                               trace=_trace, **(_trace_kwargs or {}))
    _CACHE["last_result"] = res

    out = np.empty((B, T, C), dtype=np.float32)
    for core in range(NCORES):
        b, hg = core // 4, core % 4
        out[b, :, hg * FV:(hg + 1) * FV] = res.results[core]["out"]
    return out


# revision 20
# speedup vs baseline: 1.0469x; 1.0469x over previous
"""Causal self-attention (B=2, T=2048, C=1024, H=16) on 8 TRN2 NeuronCores.

Sharding: data parallel over batch (2) x tensor parallel over heads (4 groups
of 4 heads). Each core computes qkv + attention for its 4 heads of one batch,
normalized attention outputs are AllGathered (one AG per 512-row t-chunk, the
last chunk per-pair) within each batch group of 4 cores, and each core then
computes a 256-column slice of the output projection. The host concatenates
the column slices (pure gather, no reduction).

v4: the causal boundary mask is applied additively (-1e8) by the tensor
engine into the scores PSUM via an identity matmul, removing the
exp->vector->AV serialization; startup loads at half-tile granularity across
three trigger queues; qkv/proj matmuls pumped into the exp-rate-bound
attention loop as fillers; per-pair AllGather for the tail chunk with
pair-major proj so the tail chases the final collective.
"""
import numpy as np
import ml_dtypes

import concourse.bass as bass
import concourse.tile as tile
from concourse import bacc, mybir
from concourse.bass_utils import run_bass_kernel_spmd

BF16 = ml_dtypes.bfloat16

B, T, C, H, D = 2, 2048, 1024, 16, 64
NCORES = 8
HPC = 4              # heads per core
FQK = 2 * HPC * D    # 512 rows of q+k per core
FV = HPC * D         # 256 rows of v per core
CT = C // 128        # 8 contraction tiles
TC5 = T // 512       # 4 t-chunks of 512
SB = T // 128        # 16 s-blocks of 128
SCALE = 1.0 / 8.0    # 1/sqrt(D)

_CACHE = {}


def _build_kernel():
    nc = bacc.Bacc("TRN2", target_bir_lowering=False, debug=False,
                   num_devices=NCORES)
    dt = mybir.dt
    f32, bf16 = dt.float32, dt.bfloat16

    xT = nc.dram_tensor("xT", [C, T], bf16, kind="ExternalInput").ap()
    wqkT = nc.dram_tensor("wqkT", [C, FQK], bf16, kind="ExternalInput").ap()
    wvT = nc.dram_tensor("wvT", [C, FV], bf16, kind="ExternalInput").ap()
    wpT = nc.dram_tensor("wpT", [C, FV], bf16, kind="ExternalInput").ap()
    bqk = nc.dram_tensor("bqk", [FQK, 1], f32, kind="ExternalInput").ap()
    bv = nc.dram_tensor("bv", [1, FV], bf16, kind="ExternalInput").ap()
    bp = nc.dram_tensor("bp", [1, FV], bf16, kind="ExternalInput").ap()
    negm = nc.dram_tensor("negm", [128, 128], bf16, kind="ExternalInput").ap()
    ident = nc.dram_tensor("ident", [128, 128], bf16, kind="ExternalInput").ap()
    onesr = nc.dram_tensor("onesr", [1, 64], mybir.dt.float32r,
                           kind="ExternalInput").ap()
    out = nc.dram_tensor("out", [T, FV], f32, kind="ExternalOutput").ap()

    with tile.TileContext(nc) as tc:
        with (
            tc.tile_pool(name="persist", bufs=1) as pp,
            tc.tile_pool(name="work", bufs=4) as wp,
            tc.tile_pool(name="attT", bufs=6) as ap_pool,
            tc.tile_pool(name="outsb", bufs=3) as op,
            tc.tile_pool(name="ps_qk", bufs=2, space="PSUM") as ps_qk,
            tc.tile_pool(name="ps_y", bufs=2, space="PSUM") as ps_y,
            tc.tile_pool(name="ps_f", bufs=2, space="PSUM") as ps_f,
            tc.tile_pool(name="dram", bufs=1, space="DRAM") as dram,
        ):
            # local constants first so the PE warmup can start immediately
            ones16 = pp.tile([1, 128], bf16, tag="ones16")
            nc.vector.memset(ones16[:], 1.0)
            junk_b = pp.tile([1, 512], bf16, tag="junk_b")
            nc.vector.memset(junk_b[:], 1.0)
            junk_f = pp.tile([1, 64], f32, tag="junk_f")
            nc.vector.memset(junk_f[:], 0.0)
            junk_o = pp.tile([1, 64], bf16, tag="junk_o")

            wqk_s = pp.tile([128, CT, FQK], bf16, tag="wqk")
            xT_s = pp.tile([128, CT, T], bf16, tag="xT")
            wv_s = pp.tile([128, CT, FV], bf16, tag="wv")
            wp_s = pp.tile([128, CT, FV], bf16, tag="wp")
            bqk_s = pp.tile([128, 4], f32, tag="bqk")
            bv_s = pp.tile([1, FV], bf16, tag="bv")
            bp_s = pp.tile([1, FV], bf16, tag="bp")
            negm_s = pp.tile([128, 128], bf16, tag="negm")
            ident_s = pp.tile([128, 128], bf16, tag="ident")
            ones32 = pp.tile([1, 64], mybir.dt.float32r, tag="ones32")

            xTr = xT.rearrange("(n p) t -> p n t", p=128)
            wqkr = wqkT.rearrange("(n p) f -> p n f", p=128)
            wvr = wvT.rearrange("(n p) f -> p n f", p=128)
            wpr = wpT.rearrange("(n p) f -> p n f", p=128)

            # critical loads at half-tile granularity: each dma_start lands
            # on one ~13GB/s DMA engine, so more triggers = more parallelism.
            # sync: wqk halves; scalar: wv first (v chains follow qkT by only
            # ~10us), then x chunk 0, then wp; gpsimd (SWDGE): small consts,
            # then the later x chunks (naturally delayed behind the
            # startup-critical traffic by the ~1us/trigger SWDGE issue rate).
            for ci in range(0, CT, 2):
                nc.scalar.dma_start(wv_s[:, ci:ci + 2, :], wvr[:, ci:ci + 2, :])
            for ci in range(CT):
                nc.sync.dma_start(wqk_s[:, ci, 0:256], wqkr[:, ci, 0:256])
                nc.sync.dma_start(wqk_s[:, ci, 256:512], wqkr[:, ci, 256:512])
                nc.scalar.dma_start(xT_s[:, ci, 0:256], xTr[:, ci, 0:256])
                nc.scalar.dma_start(xT_s[:, ci, 256:512], xTr[:, ci, 256:512])
            nc.gpsimd.dma_start(bqk_s[:], bqk.rearrange("(n p) o -> p (n o)", p=128))
            nc.gpsimd.dma_start(bv_s[:], bv[:])
            nc.gpsimd.dma_start(bp_s[:], bp[:])
            nc.gpsimd.dma_start(ones32[:], onesr[:])
            nc.gpsimd.dma_start(negm_s[:], negm[:])
            nc.gpsimd.dma_start(ident_s[:], ident[:])
            # later x chunks on the two HWDGE queues, in the order the
            # front-loaded qkv chains consume them
            for ci in range(CT):
                nc.sync.dma_start(xT_s[:, ci, 512:1024], xTr[:, ci, 512:1024])
            for ci in range(CT):
                nc.scalar.dma_start(xT_s[:, ci, 1024:1536],
                                    xTr[:, ci, 1024:1536])
            for ci in range(CT):
                nc.sync.dma_start(xT_s[:, ci, 1536:2048], xTr[:, ci, 1536:2048])
            for ci in range(0, CT, 4):
                nc.scalar.dma_start(wp_s[:, ci:ci + 4, :], wpr[:, ci:ci + 4, :])

            # ---- warm the collective stream: a tiny AllGather issued at
            #      startup absorbs the ~11us first-collective latency (and
            #      queues behind the runtime's skew barrier) ----
            agw_in = dram.tile([1, 64], bf16, tag="agw_i", name="agw_i")
            agw_out = dram.tile([4, 64], bf16, tag="agw_o", name="agw_o")
            nc.gpsimd.dma_start(agw_in[:], bv[0:1, 0:64])
            nc.gpsimd.collective_compute(
                "AllGather", mybir.AluOpType.bypass,
                replica_groups=[[0, 1, 2, 3], [4, 5, 6, 7]],
                ins=[agw_in[:].opt()],
                outs=[agw_out[:].opt()],
            )

            # ---- PE p-state warmup + ACT table warm ----
            psw = ps_f.tile([128, 512], f32, tag="f", name="ps_warm")
            for i in range(6):
                nc.tensor.matmul(psw[:], ones16[0:1, :], junk_b[0:1, :],
                                 start=(i == 0), stop=(i == 5))
            nc.scalar.activation(junk_o[:], junk_f[:],
                                 mybir.ActivationFunctionType.Exp, scale=SCALE)

            bv_bc = pp.tile([128, FV], f32, tag="bv_bc")
            bp_bc = pp.tile([128, FV], f32, tag="bp_bc")

            def gen_bias_bc():
                # broadcast bias rows to all 128 partitions (K=1 matmuls)
                for row, bc_t in ((bv_s, bv_bc), (bp_s, bp_bc)):
                    psb = ps_f.tile([128, 512], f32, tag="f", name="ps_bias")
                    nc.tensor.matmul(psb[:, 0:FV], ones16[0:1, :], row[0:1, :],
                                     start=True, stop=True)
                    nc.vector.tensor_copy(bc_t[:], psb[:, 0:FV])
                    yield

            # ---- qkT / v storage ----
            qkT_s = [pp.tile([128, T], bf16, tag=f"qkT{fc}", name=f"qkT{fc}")
                     for fc in range(4)]
            vaug = pp.tile([128, SB, HPC * 65], bf16, tag="vaug")
            # only the per-head "ones" columns (col 64 of each 65) need init
            vones = vaug[:].rearrange("p s (h x) -> p s h x", h=HPC)[:, :, :, 64:65]
            nc.vector.memset(vones, 1.0)

            # ---- filler emitters: single-matmul granularity ----
            def gen_qkT_tile(t5, fc):
                ps = ps_f.tile([128, 512], f32, tag="f", name="ps_qkv")
                if t5 == 0:
                    # chunk 0 streams in as [128,256] halves; match that.
                    # start=True zeroes the WHOLE psum bank, so only the very
                    # first matmul may carry it — the second region then
                    # accumulates onto the zeroed bank.
                    for j, (ci, th) in enumerate(
                            (ci, th) for ci in range(CT) for th in range(2)):
                        nc.tensor.matmul(
                            ps[:, th * 256:(th + 1) * 256],
                            wqk_s[:, ci, fc * 128:(fc + 1) * 128],
                            xT_s[:, ci, th * 256:(th + 1) * 256],
                            start=(j == 0), stop=(ci == CT - 1),
                            skip_group_check=True,
                        )
                        if j < 2 * CT - 1:
                            yield
                else:
                    for ci in range(CT):
                        nc.tensor.matmul(
                            ps[:],
                            wqk_s[:, ci, fc * 128:(fc + 1) * 128],
                            xT_s[:, ci, t5 * 512:(t5 + 1) * 512],
                            start=(ci == 0), stop=(ci == CT - 1),
                        )
                        if ci < CT - 1:
                            yield
                nc.vector.tensor_scalar_add(
                    qkT_s[fc][:, t5 * 512:(t5 + 1) * 512], ps[:],
                    bqk_s[:, fc:fc + 1],
                )
                yield

            def gen_v_tile(tb):
                ps = ps_f.tile([128, 512], f32, tag="f", name="ps_v")
                for ci in range(CT):
                    nc.tensor.matmul(
                        ps[:, 0:FV],
                        xT_s[:, ci, tb * 128:(tb + 1) * 128],
                        wv_s[:, ci, :],
                        start=(ci == 0), stop=(ci == CT - 1),
                    )
                    if ci < CT - 1:
                        yield
                dst = vaug[:, tb, :].rearrange("p (h x) -> p h x", h=HPC)[:, :, 0:64]
                src = ps[:, 0:FV].rearrange("p (h x) -> p h x", h=HPC)
                bias = bv_bc[:].rearrange("p (h x) -> p h x", h=HPC)
                nc.vector.scalar_tensor_tensor(
                    dst, src, 1.0, bias,
                    op0=mybir.AluOpType.mult, op1=mybir.AluOpType.add,
                )
                yield

            yf = {}

            def gen_proj_chunk(t5):
                # one psum chain per 128-row output tile; the tail chunk runs
                # pair-0 contraction tiles first so only the back half of each
                # chain waits on AG(3,1)
                yft = yf[t5]
                ci_order = (0, 2, 4, 6, 1, 3, 5, 7) if t5 == 3 else range(CT)
                for tq in range(4):
                    pso = ps_f.tile([128, 512], f32, tag="f", name="ps_o")
                    for j, ci in enumerate(ci_order):
                        par, cc = ci % 2, ci // 2
                        nc.tensor.matmul(
                            pso[:, 0:FV],
                            yft[:, cc, par * 512 + tq * 128:
                                par * 512 + (tq + 1) * 128],
                            wp_s[:, ci, :],
                            start=(j == 0), stop=(j == CT - 1),
                            skip_group_check=True,
                        )
                        if j < CT - 1:
                            yield
                    tb = t5 * 4 + tq
                    osb = op.tile([128, FV], f32, tag="osb", name="osb")
                    nc.vector.tensor_add(osb[:], pso[:, 0:FV], bp_bc[:])
                    nc.sync.dma_start(out[tb * 128:(tb + 1) * 128, :], osb[:])
                    yield

            fillers = []        # FIFO of single-matmul generators

            def pump(n=1):
                while n > 0 and fillers:
                    try:
                        next(fillers[0])
                        n -= 1
                    except StopIteration:
                        fillers.pop(0)

            def drain():
                while fillers:
                    pump(64)

            def drain_gens(gens):
                while any(g in fillers for g in gens):
                    pump(8)

            # ---- chunk 0 qkv up front; later chunks pump into attention
            for fc in (0, 2):
                fillers.append(gen_qkT_tile(0, fc))
            fillers.append(gen_bias_bc())
            for tb in range(4):
                fillers.append(gen_v_tile(tb))
            drain()
            g13 = [gen_qkT_tile(0, 1), gen_qkT_tile(0, 3)]
            fillers.extend(g13)

            # ---- attention loop, t-chunk major; AG + proj pipelined ----
            ag_in, ag_out = {}, {}

            def emit_attention(t5, pair):
                live = 4 * (t5 + 1)
                q_fc, k_fc = pair, 2 + pair
                ypsA = ps_y.tile([65, 512], f32, tag="y", name="ypsA")
                ypsB = ps_y.tile([65, 512], f32, tag="y", name="ypsB")
                for sb in range(live):
                    off = sb * 128 - t5 * 512
                    w = 512 - max(off, 0)     # live columns of this t-chunk
                    ps = ps_qk.tile([128, 1024], f32, tag="qk", name="ps_s")
                    bdry = off >= 0
                    for hh in range(2):
                        lo, hi = 64 * hh, 64 * (hh + 1)
                        nc.tensor.matmul(
                            ps[:, hh * 512 + 512 - w:(hh + 1) * 512],
                            qkT_s[k_fc][lo:hi, sb * 128:(sb + 1) * 128],
                            qkT_s[q_fc][lo:hi,
                                        (t5 + 1) * 512 - w:(t5 + 1) * 512],
                            start=True, stop=not bdry,
                            skip_group_check=True,
                        )
                    if bdry:
                        # additive causal mask (-1e8 above the diagonal) into
                        # the boundary 128 columns, via identity matmul
                        o = max(off, 0)
                        for hh in range(2):
                            nc.tensor.matmul(
                                ps[:, hh * 512 + o:hh * 512 + o + 128],
                                ident_s[:], negm_s[:],
                                start=False, stop=True,
                                skip_group_check=True,
                            )
                    a = ap_pool.tile([128, 1024], bf16, tag="attT",
                                     name="attT")
                    av = a[:].rearrange("p (g x) -> p g x", g=2)
                    pv = ps[:].rearrange("p (g x) -> p g x", g=2)
                    if off > 0:
                        nc.scalar.activation(
                            av[:, :, off:512], pv[:, :, off:512],
                            mybir.ActivationFunctionType.Exp, scale=SCALE,
                        )
                    else:
                        nc.scalar.activation(
                            a[:], ps[:],
                            mybir.ActivationFunctionType.Exp, scale=SCALE,
                        )
                    pump(1 if (off > 0 or sb % 2) else 2)
                    o = max(off, 0)
                    for hh, yps in ((0, ypsA), (1, ypsB)):
                        h = pair * 2 + hh
                        nc.tensor.matmul(
                            yps[:, o:512],
                            vaug[:, sb, h * 65:(h + 1) * 65],
                            a[:, hh * 512 + o:(hh + 1) * 512],
                            start=(sb == 0), stop=(sb == live - 1),
                            skip_group_check=True,
                        )
                # normalize: y / denom (denom = row 64 via ones column)
                for hh, yps in ((0, ypsA), (1, ypsB)):
                    den = wp.tile([1, 512], mybir.dt.float32r, tag="den",
                                  name="den")
                    nc.vector.tensor_copy(den[:], yps[64:65, :])
                    bc = ps_f.tile([128, 512], f32, tag="f", name="bc")
                    nc.tensor.matmul(bc[0:64, :], ones32[0:1, :], den[:],
                                     start=True, stop=True)
                    r = wp.tile([64, 512], f32, tag="recip", name="recip")
                    nc.vector.reciprocal_approx_fast(r[:], bc[0:64, :])
                    yn = wp.tile([64, 512], bf16, tag="yn", name="yn")
                    nc.vector.tensor_mul(yn[:], yps[0:64, :], r[:])
                    if t5 == 3:
                        nc.sync.dma_start(
                            ag_in[(3, pair)][hh * 64:(hh + 1) * 64, 0:256],
                            yn[:, 0:256])
                        nc.scalar.dma_start(
                            ag_in[(3, pair)][hh * 64:(hh + 1) * 64, 256:512],
                            yn[:, 256:512])
                    else:
                        nc.sync.dma_start(
                            ag_in[t5][hh * 64:(hh + 1) * 64,
                                      pair * 512:(pair + 1) * 512], yn[:])

            def trigger_ag(key, width):
                ag_out[key] = dram.tile([512, width], bf16,
                                        tag=f"agout{key}", name=f"agout{key}")
                nc.gpsimd.collective_compute(
                    "AllGather", mybir.AluOpType.bypass,
                    replica_groups=[[0, 1, 2, 3], [4, 5, 6, 7]],
                    ins=[ag_in[key][:].opt()],
                    outs=[ag_out[key][:].opt()],
                )

            def pull_yf(t5, eng):
                # [128,512] pieces (~10us each on one DMA engine, parallel
                # across engines); emitted as soon as AG(t5) is sure to finish
                # before anything queued behind these on `eng` is needed
                for cc in range(4):
                    for ph in range(2):
                        eng.dma_start(
                            yf[t5][:, cc, ph * 512:(ph + 1) * 512],
                            ag_out[t5][cc * 128:(cc + 1) * 128,
                                       ph * 512:(ph + 1) * 512])

            for t5 in range(TC5):
                yf[t5] = pp.tile([128, 4, 1024], bf16, tag="yf",
                                 bufs=3, name=f"yf{t5}")
                if t5 == 3:
                    for pair in range(2):
                        ag_in[(3, pair)] = dram.tile(
                            [128, 512], bf16, tag=f"agin3_{pair}",
                            name=f"agin3_{pair}")
                else:
                    ag_in[t5] = dram.tile([128, 1024], bf16, tag=f"agin{t5}",
                                          name=f"agin{t5}")
                # queue fillers: next chunk's qkv; late chunks add proj work
                if t5 + 1 < TC5:
                    for fc in range(4):
                        fillers.append(gen_qkT_tile(t5 + 1, fc))
                    for tb in range(4 * (t5 + 1), 4 * (t5 + 1) + 4):
                        fillers.append(gen_v_tile(tb))
                if t5 == 2:
                    pull_yf(0, nc.sync)       # AG(0) long done
                if t5 == 3:
                    fillers.append(gen_proj_chunk(1))
                emit_attention(t5, 0)
                if t5 == 0:
                    drain_gens(g13)
                if t5 == 2:
                    pull_yf(1, nc.sync)       # AG(1) done ~20us ago
                    fillers.append(gen_proj_chunk(0))
                if t5 == 3:
                    trigger_ag((3, 0), 512)
                    pull_yf(2, nc.gpsimd)     # AG(2) completed during pair 0
                    fillers.append(gen_proj_chunk(2))
                emit_attention(t5, 1)
                if t5 == 3:
                    trigger_ag((3, 1), 512)
                else:
                    trigger_ag(t5, 1024)
                drain()
            # tail: pair-0 pull on gpsimd (AG(3,0) done during pair-1
            # attention); pair-1 in 64KB pieces split across sync+scalar
            # (both idle at the tail), chasing AG(3,1)
            for cc in range(4):
                nc.gpsimd.dma_start(yf[3][:, cc, 0:512],
                                    ag_out[(3, 0)][cc * 128:(cc + 1) * 128, :])
            for cc in range(4):
                for ph, eng in ((0, nc.sync), (1, nc.scalar)):
                    eng.dma_start(
                        yf[3][:, cc, 512 + ph * 256:512 + (ph + 1) * 256],
                        ag_out[(3, 1)][cc * 128:(cc + 1) * 128,
                                       ph * 256:(ph + 1) * 256])
            fillers.append(gen_proj_chunk(3))
            drain()

    nc.compile()
    return nc


def _shard_inputs(x, w_attn, b_attn, w_proj, b_proj):
    negm = np.zeros((128, 128), dtype=BF16)
    for s in range(128):
        negm[s, :s] = -800.0   # -100 after scale: exp()->0 within table range
    ident = np.eye(128, dtype=BF16)

    in_maps = []
    for core in range(NCORES):
        b, hg = core // 4, core % 4
        r0 = hg * HPC * D          # first q/k/v row offset within each 1024
        r1 = r0 + HPC * D
        wqk = np.concatenate([w_attn[r0:r1, :], w_attn[C + r0:C + r1, :]], 0)
        in_maps.append({
            "xT": np.ascontiguousarray(x[b].T).astype(BF16),
            "wqkT": np.ascontiguousarray(wqk.T).astype(BF16),
            "wvT": np.ascontiguousarray(w_attn[2 * C + r0:2 * C + r1, :].T).astype(BF16),
            "wpT": np.ascontiguousarray(w_proj[r0:r1, :].T).astype(BF16),
            "bqk": np.concatenate([b_attn[r0:r1], b_attn[C + r0:C + r1]])
                     .reshape(FQK, 1).astype(np.float32),
            "bv": b_attn[2 * C + r0:2 * C + r1].reshape(1, FV).astype(BF16),
            "bp": b_proj[r0:r1].reshape(1, FV).astype(BF16),
            "negm": negm,
            "ident": ident,
            "onesr": np.ones((1, 64), dtype=np.float32),
        })
    return in_maps


def kernel(x, w_attn, b_attn, w_proj, b_proj, _trace=False, _trace_kwargs=None):
    x = np.asarray(x, dtype=np.float32)
    w_attn = np.asarray(w_attn, dtype=np.float32)
    b_attn = np.asarray(b_attn, dtype=np.float32)
    w_proj = np.asarray(w_proj, dtype=np.float32)
    b_proj = np.asarray(b_proj, dtype=np.float32)

    if "nc" not in _CACHE:
        _CACHE["nc"] = _build_kernel()
    nc = _CACHE["nc"]

    in_maps = _shard_inputs(x, w_attn, b_attn, w_proj, b_proj)
    res = run_bass_kernel_spmd(nc, in_maps, core_ids=list(range(NCORES)),
                               trace=_trace, **(_trace_kwargs or {}))
    _CACHE["last_result"] = res

    out = np.empty((B, T, C), dtype=np.float32)
    for core in range(NCORES):
        b, hg = core // 4, core % 4
        out[b, :, hg * FV:(hg + 1) * FV] = res.results[core]["out"]
    return out
